# revision 1
# baseline (speedup 1.0000x reference)
"""Trainium2 Bass kernel for single-head self-attention over image tokens.

Reference computation (per batch element b of 4):
    xf   = x[b] viewed as [N=4096 tokens, C=256]          (x stored [C, H*W] = xf.T)
    qkv  = xf @ w_qkv.T                                   -> q, k, v each [N, 512]
    sim  = (q * 64**-0.5) @ k.T                           [N, N]
    attn = softmax(sim, axis=-1)
    out  = (attn @ v) @ w_out.T + b_out + xf              [N, C]

Sharding: 8 cores = 4 batches x 2 query-row halves (2048 rows each). Each core
computes k/v for its full batch but q/out only for its half. No collectives.
Each core's x is host-rotated so its query half is always columns 0:2048
(softmax over keys is permutation invariant, so key order doesn't matter).

Mixed precision, tuned against the 2e-2 rel-err budget (measured 1.83e-2,
deterministic):
  - QKV projection: x and w_qkv in bf16 (halves the input DMA), f32 PSUM.
  - sim contraction (512 dims): dims 0:256 as an fp8e4 DoubleRow matmul
    (2 fp8 weights/cell, double ALU rate), dims 256:512 in bf16 (bf16 mms
    issue ~15 cycles faster than float32r's FP32_HIGH mode and get fast
    weight load). Full-fp8 q/k alone measures 2.03e-2 -- over budget -- so
    only half the dims go fp8.
  - pT = exp(0.125*sim - 7): fp8e5 (e5m2's 22-octave range covers the
    per-query softmax max spread; e4m3 flushes weak queries to zero). The
    shift cancels in out = po/l. attn@v runs as fp8 DoubleRow over
    token-pair planes (v in fp8e4), f32 PSUM.
  - out projection in bf16; residual read from the resident bf16 x.

On-chip layout keeps everything in the "transposed activation" orientation so
no PE transposes are needed: qT/kT come straight out of the QKV projection
(x's HBM layout [C, N] is already the rhs/lhsT the PE wants), v [N, 512] uses
x slices as the stationary operand. simT [j, i] accumulates per 128-token
chunk, outT [d, i] += v.T @ pT per 1024-column j-superblock in PSUM, and the
softmax denominator is tree-summed on the DVE (deferred ones-matmul) except
on the final slice, where it rides the PE as a DoubleRow ones-matmul so the
kernel tail never waits on the adder tree. Finalizes are deferred into the
next slice's PE stream; pb/pf reuse the aux/sim PSUM banks so slice
boundaries don't stall on PSUM; the final slice's finalize is split into two
256-col halves with output DMAs spread across the sync/scalar/gpsimd queues.
Startup: weights (q/k/v split, sync queue) and x (512-col chunks, gpsimd
queue) stream in parallel; jb0/jb1 and the residual reuse the resident
q-phase x chunks instead of re-reading x.
"""

import hashlib
import os
import shutil

import numpy as np

import concourse.bacc as bacc
import concourse.tile as tile
import concourse.mybir as mybir
from concourse.bass_utils import run_bass_kernel_spmd


def _install_neff_cache():
    """Disk-cache walrus NEFF compiles keyed on the BIR content hash.

    The axon PJRT path recompiles the NEFF in every fresh process (~minutes);
    the build here is deterministic, so identical BIR -> identical NEFF.
    """
    try:
        import concourse.bass2jax as bass2jax
        orig = bass2jax.compile_bir_kernel
        if getattr(orig, "_neff_cache_wrapped", False):
            return
        cache_dir = os.path.expanduser("~/.neuron-compile-cache/bass-neff")

        def cached(bir_json, tmpdir, neff_name="file.neff"):
            try:
                key = hashlib.sha256(
                    bir_json if isinstance(bir_json, bytes)
                    else bir_json.encode()).hexdigest()
                hit = os.path.join(cache_dir, key + ".neff")
                dst = os.path.join(tmpdir, neff_name)
                if os.path.exists(hit):
                    shutil.copyfile(hit, dst)
                    return dst
                neff = orig(bir_json, tmpdir, neff_name=neff_name)
                os.makedirs(cache_dir, exist_ok=True)
                tmp = hit + ".tmp%d" % os.getpid()
                shutil.copyfile(neff, tmp)
                os.replace(tmp, hit)
                return neff
            except Exception:
                return orig(bir_json, tmpdir, neff_name=neff_name)

        cached._neff_cache_wrapped = True
        bass2jax.compile_bir_kernel = cached
    except Exception:
        pass


_install_neff_cache()

F32 = mybir.dt.float32
F32R = mybir.dt.float32r
BF16 = mybir.dt.bfloat16
F8E4 = mybir.dt.float8e4
F8E5 = mybir.dt.float8e5
DR = mybir.MatmulPerfMode.DoubleRow
Exp = mybir.ActivationFunctionType.Exp
SHIFT = 7.0  # exp(scale*sim - SHIFT): keeps pT <= e^10.1 < e5m2 max 57344;
             # cancels in out = po/l so no renormalization needed

B = 4
C = 256          # model dim (2 chunks of 128)
N = 4096         # tokens per batch (64*64)
HALF = N // 2    # query rows per core
INNER = 512      # qkv inner dim (4 chunks of 128)
SCALE = 0.125    # 64 ** -0.5

NCORES = 8
NJB = 4          # j superblocks per batch
JBW = N // NJB   # 1024 key columns per superblock
NSL = 4          # i slices per core
SW = HALF // NSL # 512 query columns per slice


def build_nc(n=N, njb=NJB, nsl=NSL):
    half = n // 2
    jbw = n // njb
    assert half % SW == 0 and jbw % SW == 0 and jbw % 256 == 0
    nc = bacc.Bacc(None)
    x_r = nc.declare_dram_parameter("x_r", [C, n], BF16, isOutput=False)
    wqkvT = nc.declare_dram_parameter("wqkvT", [C, 3 * INNER], BF16, isOutput=False)
    woutT = nc.declare_dram_parameter("woutT", [INNER, C], BF16, isOutput=False)
    bout = nc.declare_dram_parameter("bout", [2, 128, 1], F32, isOutput=False)
    out = nc.declare_dram_parameter("out", [C, half], F32, isOutput=True)

    mm = nc.tensor.matmul

    with tile.TileContext(nc) as tc:
        with tc.tile_pool(name="const", bufs=1) as const, \
             tc.tile_pool(name="stream", bufs=1) as stream, \
             tc.tile_pool(name="work", bufs=2) as work, \
             tc.tile_pool(name="pp", bufs=1, space="PSUM") as pp:

            # ---- resident weights: bf16 DMA, split per q/k/v so the
            # first qT matmul only waits on the q part
            wqq, wqk, wqv = [], [], []
            for part, lst in ((0, wqq), (1, wqk), (2, wqv)):
                for cc in range(2):
                    t = const.tile([128, INNER], BF16, tag=f"wq{part}{cc}",
                                   name=f"wq{part}{cc}")
                    nc.sync.dma_start(
                        t, wqkvT[cc * 128:(cc + 1) * 128,
                                 part * INNER:(part + 1) * INNER])
                    lst.append(t)

            def xchunk(cc, col, width):
                """x chunk [128, width] bf16, shares slots with xjb tiles.
                GpSimd-queue DMA: runs in parallel with the weight stream on
                the sync queue."""
                t = stream.tile([128, width], BF16, tag=f"xjb{cc}", bufs=6,
                                name=f"xjb{cc}", padded_shape=[128, jbw])
                nc.gpsimd.dma_start(t, x_r[cc * 128:(cc + 1) * 128, col:col + width])
                return t

            qTp = const.tile([128, 2, half], F8E4, tag="qtp", name="qtp")
            qT23 = [const.tile([128, half], BF16, tag=f"qt{d}", name=f"qt{d}")
                    for d in range(2)]
            ot = [const.tile([128, half], F32, tag=f"ot{d}", name=f"ot{d}")
                  for d in range(4)]
            l_sb = const.tile([1, half], F32, tag="l_sb", name="l_sb")

            ones_col_f = const.tile([128, 1], F32, tag="ones_col_f", name="ones_col_f")
            nc.vector.memset(ones_col_f, 1.0)
            ones_col = const.tile([128, 1], F32R, tag="ones_col", name="ones_col")
            nc.vector.tensor_copy(ones_col, ones_col_f)
            ones_row_f = const.tile([1, 128], F32, tag="ones_row_f", name="ones_row_f")
            nc.vector.memset(ones_row_f, 1.0)
            ones_row = const.tile([1, 128], F32R, tag="ones_row", name="ones_row")
            nc.vector.tensor_copy(ones_row, ones_row_f)
            nshift = const.tile([128, 1], F32, tag="nshift", name="nshift")
            nc.vector.memset(nshift, -SHIFT)
            # [128, 2, 16] so the pair-plane stride is 16 B (dual-fp8
            # ldweights ISA restriction); only column 0 is used
            ones_pair_f = const.tile([128, 2, 16], F32, tag="ones_pair_f",
                                     name="ones_pair_f")
            nc.vector.memset(ones_pair_f, 1.0)
            ones_pair = const.tile([128, 2, 16], F8E4, tag="ones_pair",
                                   name="ones_pair")
            nc.vector.tensor_copy(ones_pair, ones_pair_f)
            ones_sq_f = const.tile([128, 128], F32, tag="ones_sq_f",
                                   name="ones_sq_f")
            nc.vector.memset(ones_sq_f, 1.0)
            ones_sq = const.tile([128, 128], F32R, tag="ones_sq", name="ones_sq")
            nc.vector.tensor_copy(ones_sq, ones_sq_f)

            # ---- qT production from x columns 0:half ----
            wo = []
            bt = []
            qcw = SW  # small first blocks: compute starts after ~0.75MB of DMA
            x_qch = []  # 512-wide x chunks over cols 0:half; reused by jb 0/1
                        # and as the residual at finalize (bufs=6 keeps all
                        # four resident alongside jb2/jb3's chunks)
            for qch in range(half // qcw):
                xch = [xchunk(cc, qch * qcw, qcw) for cc in range(2)]
                x_qch.append(xch)
                for d in range(4):
                    ps = pp.tile([128, SW], F32, tag="sim", bufs=3, name="ps_q")
                    for cc in range(2):
                        mm(ps, wqq[cc][:, d * 128:(d + 1) * 128],
                           xch[cc][:, 0:SW],
                           start=(cc == 0), stop=(cc == 1))
                    if d < 2:
                        nc.scalar.copy(
                            qTp[:, d, qch * SW:(qch + 1) * SW], ps)
                    else:
                        nc.scalar.copy(
                            qT23[d - 2][:, qch * SW:(qch + 1) * SW], ps)
            # final-phase constants, off the startup critical path
            # (vector-queue DMAs so the sync queue stays free for x chunks)
            for d in range(4):
                t = const.tile([128, C], BF16, tag=f"wo{d}", name=f"wo{d}")
                nc.scalar.dma_start(t, woutT[d * 128:(d + 1) * 128, :])
                wo.append(t)
            for cc in range(2):
                t = const.tile([128, 1], F32, tag=f"b{cc}", name=f"b{cc}")
                nc.scalar.dma_start(t, bout[cc])
                bt.append(t)

            # ---- attention over j superblocks ----
            deferred = []   # denominator work deferred into later PE streams
            deferred2 = []  # second-stage finalize work (flushed at j8==4)
            for jb in range(njb):
                if jb < 2:
                    # cols jb*jbw : (jb+1)*jbw are the query half: reuse the
                    # resident q-phase chunks instead of re-reading x
                    nq = jbw // qcw

                    def xap(cc, start, width, jb=jb, nq=nq):
                        col = jb * jbw + start
                        return x_qch[col // qcw][cc][:, col % qcw:
                                                     col % qcw + width]
                else:
                    xjb = [xchunk(cc, jb * jbw, jbw) for cc in range(2)]

                    def xap(cc, start, width, xjb=xjb):
                        return xjb[cc][:, start:start + width]
                # kT for this superblock: [512, jbw]; d-chunks 0,1 in an
                # fp8e4 pair plane (DoubleRow sim), 2,3 in f32r
                ktp = stream.tile([128, 2, jbw], F8E4, tag="ktp", bufs=1,
                                  name="ktp")
                kt23 = [stream.tile([128, jbw], BF16, tag=f"kt{d}", bufs=1,
                                    name=f"kt{d}") for d in range(2)]
                for d in range(4):
                    for nb in range(jbw // SW):
                        ps = pp.tile([128, SW], F32, tag="sim", bufs=3,
                                     name="ps_k")
                        for cc in range(2):
                            mm(ps, wqk[cc][:, d * 128:(d + 1) * 128],
                               xap(cc, nb * SW, SW),
                               start=(cc == 0), stop=(cc == 1))
                        if d < 2:
                            nc.scalar.copy(
                                ktp[:, d, nb * SW:(nb + 1) * SW], ps)
                        else:
                            nc.scalar.copy(
                                kt23[d - 2][:, nb * SW:(nb + 1) * SW], ps)
                # v: [jbw, 512] (token rows on partitions), fp8e4
                # token-pair planes for DoubleRow po matmuls
                vtp = []
                for t2 in range(jbw // 256):
                    t = stream.tile([128, 2, INNER], F8E4, tag=f"vt{t2}",
                                    bufs=1, name=f"vt{t2}")
                    vtp.append(t)
                for nj in range(jbw // 128):
                    ps = pp.tile([128, INNER], F32, tag="sim", bufs=3,
                                 name="ps_v")
                    for cc in range(2):
                        mm(ps, xap(cc, nj * 128, 128),
                           wqv[cc][:, :],
                           start=(cc == 0), stop=(cc == 1))
                    nc.scalar.copy(vtp[nj // 2][:, nj % 2, :], ps)

                for fn in deferred:
                    fn()
                deferred.clear()
                for fn in deferred2:
                    fn()
                deferred2.clear()

                nj8 = jbw // 128
                npair = nj8 // 2
                for s in range(nsl):
                    sl = slice(s * SW, (s + 1) * SW)
                    last_jb = jb == njb - 1
                    po = [pp.tile([128, SW], F32, tag=f"po{d}", bufs=1,
                                  name=f"po{d}") for d in range(4)]
                    tail_slice = last_jb and s == nsl - 1
                    if not last_jb or tail_slice:
                        pl = pp.tile([1, SW], F32, tag="aux", bufs=1, name="pl")
                    else:
                        pl = None
                    if last_jb:
                        # l through jb 0..2 in f32r, off the critical path
                        l_rs = work.tile([1, SW], F32R, tag="l_rs", bufs=2,
                                         name="l_rs")
                        nc.scalar.copy(l_rs, l_sb[:, sl])
                    ptp = []

                    sums = []  # binary tree of pT pair-sums (DVE)

                    def tree_add(t):
                        sums.append([t, 0])
                        while len(sums) >= 2 and sums[-1][1] == sums[-2][1]:
                            a, lv = sums.pop()
                            b, _ = sums.pop()
                            t2 = work.tile([128, SW], F32R, tag="pt2", bufs=4,
                                           name="pt2")
                            nc.vector.tensor_add(t2, b, a)
                            sums.append([t2, lv + 1])

                    def l_update(jb=jb, sl=sl, pl=pl):
                        if jb == 0:
                            nc.vector.tensor_copy(l_sb[:, sl], pl)
                        else:
                            nc.vector.tensor_add(l_sb[:, sl], l_sb[:, sl], pl)

                    def drain_pair(p):
                        # outT + denominator work for token-pair p (emitted a
                        # pair late so the PE never waits on the exp). po is a
                        # DoubleRow fp8 matmul: 256 tokens contracted per mm.
                        for d in range(4):
                            mm(po[d], vtp[p][:, :, d * 128:(d + 1) * 128],
                               ptp[p][:, :, :],
                               start=(p == 0), stop=(p == npair - 1),
                               perf_mode=DR)
                        if tail_slice:
                            # kernel tail: accumulate l directly on the PE
                            # with a DoubleRow ones-matmul so the finalize
                            # never waits on the DVE adder tree
                            mm(pl, ones_pair[:, :, 0:1], ptp[p][:, :, :],
                               start=(p == 0), stop=(p == npair - 1),
                               perf_mode=DR)
                            return
                        # tree-sum the pair sums on the DVE; the ones-matmul +
                        # l update (non-last jb) or the finalize (last jb) are
                        # deferred into a later PE stream so the PE never
                        # waits on the adder tree
                        pt2 = work.tile([128, SW], F32R, tag="pt2",
                                        bufs=4, name="pt2")
                        nc.vector.tensor_add(pt2, ptp[p][:, 0, :],
                                             ptp[p][:, 1, :])
                        tree_add(pt2)
                        if p == npair - 1 and not last_jb:
                            assert len(sums) == 1
                            pt8 = sums[0][0]

                            def flush(pl=pl, pt8=pt8, upd=l_update):
                                mm(pl, ones_col, pt8, start=True, stop=True)
                                upd()
                            deferred.append(flush)

                    for j8 in range(nj8):
                        ps = pp.tile([128, SW], F32, tag="sim", bufs=3, name="ps_s")
                        mm(ps, ktp[:, :, j8 * 128:(j8 + 1) * 128],
                           qTp[:, :, sl],
                           start=True, stop=False, perf_mode=DR)
                        for dd in range(2):
                            mm(ps, kt23[dd][:, j8 * 128:(j8 + 1) * 128],
                               qT23[dd][:, sl],
                               start=False, stop=(dd == 1))
                        p, parity = divmod(j8, 2)
                        if parity == 0:
                            t = work.tile([128, 2, SW], F8E5, tag="pt", bufs=4,
                                          name="pt")
                            ptp.append(t)
                        nc.scalar.activation(ptp[p][:, parity, :], ps, Exp,
                                             scale=SCALE, bias=nshift)
                        if parity == 1 and p > 0:
                            drain_pair(p - 1)
                        if j8 == 2:
                            for fn in deferred:
                                fn()
                            deferred.clear()
                        if j8 == 4:
                            for fn in deferred2:
                                fn()
                            deferred2.clear()
                    drain_pair(npair - 1)
                    if not last_jb:
                        for d in range(4):
                            if jb == 0:
                                nc.vector.tensor_copy(ot[d][:, sl], po[d])
                            else:
                                nc.vector.tensor_add(ot[d][:, sl], ot[d][:, sl],
                                                     po[d])
                    else:
                        # ---- finalize slice s: normalize + project + out ----
                        if tail_slice:
                            pt8 = None
                            l_rs3 = work.tile([1, SW], F32R, tag="l_rs3",
                                              bufs=1, name="l_rs3")
                            nc.scalar.copy(l_rs3, pl)
                        else:
                            assert len(sums) == 1
                            pt8 = sums[0][0]
                            l_rs3 = None
                        # otr = ot (jb 0..2) + po (jb 3), fused accumulate +
                        # f32r convert, emitted now so the DVE adds overlap the
                        # next slice's sim stream
                        def emit_otr(off, wdt, htag, s=s):
                            otr = [work.tile([128, wdt], BF16,
                                             tag=f"otr{htag}{d}", bufs=1,
                                             name=f"otr{htag}{d}")
                                   for d in range(4)]
                            for d in range(4):
                                nc.vector.tensor_add(
                                    otr[d],
                                    ot[d][:, s * SW + off:s * SW + off + wdt],
                                    po[d][:, off:off + wdt])
                            return otr

                        def make_finalize(off, wdt, otr, s=s, pt8=pt8,
                                          l_rs=l_rs, l_rs3=l_rs3,
                                          dma_engs=None):
                            state = {}

                            def fin_cc(cc):
                                sl2 = slice(s * SW + off, s * SW + off + wdt)
                                pf = pp.tile([128, wdt], F32, tag="sim",
                                             bufs=3, name="pf")
                                for d in range(4):
                                    mm(pf,
                                       wo[d][:, cc * 128:(cc + 1) * 128],
                                       otr[d], start=(d == 0),
                                       stop=(d == 3))
                                fo = work.tile([128, wdt], F32, tag="fo",
                                               bufs=2, name="fo")
                                nc.vector.tensor_mul(fo, pf, state["bc"])
                                # fo = (x + b) + fo: residual read straight
                                # from the resident bf16 x chunk
                                fo2 = work.tile([128, wdt], F32, tag="fo2",
                                                bufs=2, name="fo2")
                                nc.vector.scalar_tensor_tensor(
                                    fo2, x_qch[s][cc][:, off:off + wdt],
                                    bt[cc], fo,
                                    op0=mybir.AluOpType.add,
                                    op1=mybir.AluOpType.add)
                                eng = (dma_engs[cc] if dma_engs
                                       else nc.sync)
                                eng.dma_start(
                                    out[cc * 128:(cc + 1) * 128, sl2], fo2)

                            def fin_a():
                                # total l broadcast to 128 partitions in one
                                # psum accumulation: colsum(pt8) via all-ones
                                # stationary + broadcast of l_sb via ones_row
                                pb = pp.tile([128, wdt], F32, tag="aux",
                                             bufs=1, name="pb")
                                if pt8 is not None:
                                    mm(pb, ones_sq, pt8[:, off:off + wdt],
                                       start=True, stop=False)
                                else:
                                    mm(pb, ones_row, l_rs3[:, off:off + wdt],
                                       start=True, stop=False)
                                mm(pb, ones_row, l_rs[:, off:off + wdt],
                                   start=False, stop=True)
                                bc = work.tile([128, wdt], F32, tag="bc",
                                               bufs=2, name="bc")
                                rsc = work.tile([128, wdt], F32, tag="rsc",
                                                bufs=2, name="rsc")
                                nc.vector.reciprocal_approx_accurate(bc, pb,
                                                                     rsc)
                                state["bc"] = bc
                                fin_cc(0)

                            def fin_b():
                                fin_cc(1)
                            return fin_a, fin_b

                        if not tail_slice:
                            otr = emit_otr(0, SW, "")
                            fa, fb = make_finalize(0, SW, otr)
                            deferred.append(fa)
                            # second output chain lands two chunks later so
                            # the PE never waits on the DVE otr adds
                            deferred2.append(fb)
                        else:
                            # kernel tail: two 256-col halves so the first
                            # half's projection/output overlaps the second's
                            hwd = SW // 2
                            otr0 = emit_otr(0, hwd, "")
                            otr1 = emit_otr(hwd, hwd, "h")
                            a0, b0 = make_finalize(0, hwd, otr0,
                                                   dma_engs=(nc.scalar,
                                                             nc.sync))
                            a1, b1 = make_finalize(hwd, hwd, otr1,
                                                   dma_engs=(nc.gpsimd,
                                                             nc.scalar))
                            a0(); b0(); a1(); b1()

    nc.finalize()
    return nc


_NC_CACHE = None


def _get_nc():
    global _NC_CACHE
    if _NC_CACHE is None:
        _NC_CACHE = build_nc()
    return _NC_CACHE


def prepare_in_maps(x, w_qkv, w_out, b_out):
    x = np.asarray(x, dtype=np.float32)
    w_qkv = np.asarray(w_qkv, dtype=np.float32)
    w_out = np.asarray(w_out, dtype=np.float32)
    b_out = np.asarray(b_out, dtype=np.float32)

    import ml_dtypes
    bf16 = ml_dtypes.bfloat16
    xr = x.reshape(B, C, N)
    wqkvT = np.ascontiguousarray(w_qkv.T).astype(bf16)   # [C, 1536]
    woutT = np.ascontiguousarray(w_out.T).astype(bf16)   # [512, C]
    bout = np.ascontiguousarray(b_out.reshape(2, 128, 1))

    in_maps = []
    for c in range(NCORES):
        b, h = divmod(c, 2)
        if h == 0:
            x_rot = xr[b]
        else:  # rotate so this core's query half sits in columns 0:HALF
            x_rot = np.concatenate([xr[b][:, HALF:], xr[b][:, :HALF]], axis=1)
        in_maps.append({
            "x_r": x_rot.astype(bf16),
            "wqkvT": wqkvT,
            "woutT": woutT,
            "bout": bout,
        })
    return in_maps


def postprocess(results):
    outs = [results[c]["out"] for c in range(NCORES)]
    full = np.stack([np.concatenate([outs[2 * b], outs[2 * b + 1]], axis=1)
                     for b in range(B)])               # [B, C, N]
    return full.reshape(B, C, 64, 64).astype(np.float32)


def kernel(x, w_qkv, w_out, b_out):
    in_maps = prepare_in_maps(x, w_qkv, w_out, b_out)
    res = run_bass_kernel_spmd(_get_nc(), in_maps, core_ids=list(range(NCORES)))
    return postprocess(res.results)



# revision 8
# speedup vs baseline: 1.4178x; 1.4178x over previous
"""Trainium2 Bass kernel for single-head self-attention over image tokens.

Reference computation (per batch element b of 4):
    xf   = x[b] viewed as [N=4096 tokens, C=256]          (x stored [C, H*W] = xf.T)
    qkv  = xf @ w_qkv.T                                   -> q, k, v each [N, 512]
    sim  = (q * 64**-0.5) @ k.T                           [N, N]
    attn = softmax(sim, axis=-1)
    out  = (attn @ v) @ w_out.T + b_out + xf              [N, C]

Algebraic factorization (the key optimization): INNER=512 > C=256, so the
whole block collapses through two host-precomputed [256, 256] matrices
    M  = wq.T @ wk          (sim  = xf @ M @ xf.T, scaled at the exp)
    W2 = (w_out @ wv).T     (out  = softmax(...) @ xf @ W2 + b + xf)
eliminating the q/k/v projections entirely and halving both big
contractions (sim: 512 -> 256, attn@v: token-space @ xf instead of v).
Per-core PE work drops from ~432K to ~218K cycles.

Sharding: 8 cores = 4 batches x 2 query-row halves (2048 rows each). Each
core's x is host-rotated so its query half is always columns 0:2048. No
collectives.

Precision (numpy-simulated rel err 1.14e-2 vs the 2e-2 budget; the sim
predicted the previous kernel's hw error to 3 digits):
  - tT = M.T @ x: bf16 inputs, f32 PSUM, tT stored bf16.
  - sim contraction (256 dims) in bf16: stationary = resident x chunks,
    moving = tT. Full-fp8 sim measures 2.27e-2 -- over budget.
  - pT = exp(0.125*sim - 7) in fp8e5 (range); attn@xf as fp8 DoubleRow
    over token-pair planes with x in fp8e4 (host-prepared pair layout),
    f32 PSUM accumulated across ALL 4096 keys in 2 persistent PSUM banks
    per c-chunk (no SBUF accumulator, no per-superblock drain).
  - out projection W2 in bf16; residual read from the resident bf16 x.

Schedule: one pass per 512-query slice over all 32 key chunks. exp rides
the ACT engine (~14us/slice) under the PE's ~22us/slice; the softmax
denominator tree-sums alternate between DVE and Pool; each slice's
finalize (1/l via ones-matmul colsum, W2 projection, bias+residual, out
DMA) is deferred into the next slice's PE stream. The last slice handles
its final two token-pairs' denominator on the PE (DoubleRow ones-matmul)
so the kernel tail never waits on the adder tree, and finalizes in two
256-col halves with DMAs spread across queues.
"""

import hashlib
import os
import shutil

import numpy as np

import concourse.bacc as bacc
import concourse.tile as tile
import concourse.mybir as mybir
from concourse.bass_utils import run_bass_kernel_spmd


def _install_neff_cache():
    """Disk-cache walrus NEFF compiles keyed on the BIR content hash.

    The axon PJRT path recompiles the NEFF in every fresh process (~minutes);
    the build here is deterministic, so identical BIR -> identical NEFF.
    """
    try:
        import concourse.bass2jax as bass2jax
        orig = bass2jax.compile_bir_kernel
        if getattr(orig, "_neff_cache_wrapped", False):
            return
        cache_dir = os.path.expanduser("~/.neuron-compile-cache/bass-neff")

        def cached(bir_json, tmpdir, neff_name="file.neff"):
            try:
                key = hashlib.sha256(
                    bir_json if isinstance(bir_json, bytes)
                    else bir_json.encode()).hexdigest()
                hit = os.path.join(cache_dir, key + ".neff")
                dst = os.path.join(tmpdir, neff_name)
                if os.path.exists(hit):
                    shutil.copyfile(hit, dst)
                    return dst
                neff = orig(bir_json, tmpdir, neff_name=neff_name)
                os.makedirs(cache_dir, exist_ok=True)
                tmp = hit + ".tmp%d" % os.getpid()
                shutil.copyfile(neff, tmp)
                os.replace(tmp, hit)
                return neff
            except Exception:
                return orig(bir_json, tmpdir, neff_name=neff_name)

        cached._neff_cache_wrapped = True
        bass2jax.compile_bir_kernel = cached
    except Exception:
        pass


_install_neff_cache()

F32 = mybir.dt.float32
F32R = mybir.dt.float32r
BF16 = mybir.dt.bfloat16
F8E4 = mybir.dt.float8e4
F8E5 = mybir.dt.float8e5
DR = mybir.MatmulPerfMode.DoubleRow
Exp = mybir.ActivationFunctionType.Exp
Add = mybir.AluOpType.add
SHIFT = 7.0  # exp(scale*sim - SHIFT): keeps pT < e5m2 max; cancels in out

B = 4
C = 256          # model dim (2 chunks of 128)
N = 4096         # tokens per batch (64*64)
HALF = N // 2    # query rows per core
SCALE = 0.125    # 64 ** -0.5

NCORES = 8
NSL = 4          # query slices per core
SW = HALF // NSL # 512 query columns per slice
NJ8 = N // 128   # 32 key chunks
NPAIR = NJ8 // 2 # 16 key token-pairs (256 keys each)


def build_nc():
    nc = bacc.Bacc(None)
    x_r = nc.declare_dram_parameter("x_r", [C, N], BF16, isOutput=False)
    xp_d = nc.declare_dram_parameter("xp", [128, NPAIR, 2, C], F8E4,
                                     isOutput=False)
    m_d = nc.declare_dram_parameter("m", [C, C], BF16, isOutput=False)
    w2_d = nc.declare_dram_parameter("w2", [C, C], BF16, isOutput=False)
    bout = nc.declare_dram_parameter("bout", [2, 128, 1], F32, isOutput=False)
    out = nc.declare_dram_parameter("out", [C, HALF], F32, isOutput=True)

    mm = nc.tensor.matmul

    with tile.TileContext(nc) as tc:
        with tc.tile_pool(name="const", bufs=1) as const, \
             tc.tile_pool(name="work", bufs=2) as work, \
             tc.tile_pool(name="pp", bufs=1, space="PSUM") as pp:

            # ---- resident inputs ----
            # M first (tiny, sync queue) so tT mms only wait on x pieces
            mt = []
            for cc in range(2):
                t = const.tile([128, C], BF16, tag=f"m{cc}", name=f"m{cc}")
                nc.sync.dma_start(t, m_d[cc * 128:(cc + 1) * 128, :])
                mt.append(t)
            # x [C, N] bf16: 4 column pieces per c-chunk, 2 queues
            xr = [const.tile([128, N], BF16, tag=f"xr{cc}", name=f"xr{cc}")
                  for cc in range(2)]
            for piece in range(4):
                col = piece * (N // 4)
                for cc in range(2):
                    eng = nc.gpsimd if cc == 0 else nc.scalar
                    eng.dma_start(
                        xr[cc][:, col:col + N // 4],
                        x_r[cc * 128:(cc + 1) * 128, col:col + N // 4])
            # x token-pair planes fp8e4 [128, pair, plane, C], 4 pieces
            xpt = const.tile([128, NPAIR, 2, C], F8E4, tag="xpt", name="xpt")
            for piece in range(4):
                p0 = piece * (NPAIR // 4)
                nc.sync.dma_start(xpt[:, p0:p0 + NPAIR // 4, :, :],
                                  xp_d[:, p0:p0 + NPAIR // 4, :, :])

            tt = [const.tile([128, HALF], BF16, tag=f"tt{cc}", name=f"tt{cc}")
                  for cc in range(2)]

            ones_sq_f = const.tile([128, 128], F32, tag="ones_sq_f",
                                   name="ones_sq_f")
            nc.vector.memset(ones_sq_f, 1.0)
            ones_sq = const.tile([128, 128], F32R, tag="ones_sq",
                                 name="ones_sq")
            nc.vector.tensor_copy(ones_sq, ones_sq_f)
            # all-ones fp8 pair plane for the tail DoubleRow colsum
            ones_psq = const.tile([128, 2, 128], F8E4, tag="ones_psq",
                                  name="ones_psq")
            nc.gpsimd.tensor_copy(ones_psq[:, 0, :], ones_sq_f)
            nc.gpsimd.tensor_copy(ones_psq[:, 1, :], ones_sq_f)
            nshift = const.tile([128, 1], F32, tag="nshift", name="nshift")
            nc.vector.memset(nshift, -SHIFT)

            # ---- tT = M.T @ x for query columns 0:HALF ----
            ncopy = 0
            for s in range(NSL):
                sl = slice(s * SW, (s + 1) * SW)
                for co in range(2):
                    ps = pp.tile([128, SW], F32, tag="sim", bufs=3, name="ps_t")
                    for cc in range(2):
                        mm(ps, mt[cc][:, co * 128:(co + 1) * 128],
                           xr[cc][:, sl], start=(cc == 0), stop=(cc == 1))
                    # alternate the psum->bf16 copies over ACT/DVE
                    # (Pool can't access PSUM)
                    if ncopy % 2 == 0:
                        nc.scalar.copy(tt[co][:, sl], ps)
                    else:
                        nc.vector.tensor_copy(tt[co][:, sl], ps)
                    ncopy += 1
            # final-phase constants, off the startup critical path
            w2t = []
            for cc in range(2):
                t = const.tile([128, C], BF16, tag=f"w2{cc}", name=f"w2{cc}")
                nc.sync.dma_start(t, w2_d[cc * 128:(cc + 1) * 128, :])
                w2t.append(t)
            bt = []
            for cc in range(2):
                t = const.tile([128, 1], F32, tag=f"b{cc}", name=f"b{cc}")
                nc.sync.dma_start(t, bout[cc])
                bt.append(t)

            # ---- attention: one pass per query slice over all keys ----
            ew_n = 0  # DVE/Pool round-robin counter

            def ew():
                nonlocal ew_n
                ew_n += 1
                return nc.vector if ew_n % 2 else nc.gpsimd

            deferred = []   # prev slice finalize part a (into this PE stream)
            deferred2 = []  # prev slice finalize part b
            for s in range(NSL):
                sl = slice(s * SW, (s + 1) * SW)
                tail = s == NSL - 1
                po = [pp.tile([128, SW], F32, tag=f"po{cc}", bufs=2,
                              name=f"po{cc}") for cc in range(2)]
                ptp = []
                sums = []  # binary tree of pT pair-sums (DVE/Pool)

                def tree_add(t):
                    sums.append([t, 0])
                    while len(sums) >= 2 and sums[-1][1] == sums[-2][1]:
                        a, lv = sums.pop()
                        b, _ = sums.pop()
                        t2 = work.tile([128, SW], F32R, tag="pt2", bufs=6,
                                       name="pt2")
                        ew().tensor_add(t2, b, a)
                        sums.append([t2, lv + 1])

                def drain_pair(p):
                    # attn@xf for token-pair p: fp8 DoubleRow, 256 keys
                    # contracted per mm, accumulating over all 16 pairs
                    for cc in range(2):
                        mm(po[cc], xpt[:, p, :, cc * 128:(cc + 1) * 128],
                           ptp[p][:, :, :],
                           start=(p == 0), stop=(p == NPAIR - 1),
                           perf_mode=DR)
                    # denominator: pair-sum then tree-merge off the PE;
                    # the last two pairs of the last slice ride the PE
                    # instead so the tail never waits on the adder tree
                    if tail and p >= NPAIR - 2:
                        return
                    t2 = work.tile([128, SW], F32R, tag="pt2", bufs=6,
                                   name="pt2")
                    ew().tensor_add(t2, ptp[p][:, 0, :], ptp[p][:, 1, :])
                    tree_add(t2)

                for j8 in range(NJ8):
                    ps = pp.tile([128, SW], F32, tag="sim", bufs=3,
                                 name="ps_s")
                    for cc in range(2):
                        mm(ps, xr[cc][:, j8 * 128:(j8 + 1) * 128],
                           tt[cc][:, sl], start=(cc == 0), stop=(cc == 1))
                    p, parity = divmod(j8, 2)
                    if parity == 0:
                        t = work.tile([128, 2, SW], F8E5, tag="pt", bufs=4,
                                      name="pt")
                        ptp.append(t)
                    nc.scalar.activation(ptp[p][:, parity, :], ps, Exp,
                                         scale=SCALE, bias=nshift)
                    if parity == 1 and p > 0:
                        drain_pair(p - 1)
                    if j8 == 2:
                        for fn in deferred:
                            fn()
                        deferred.clear()
                    if j8 == 8:
                        for fn in deferred2:
                            fn()
                        deferred2.clear()
                drain_pair(NPAIR - 1)
                # tail: 14 tree leaves isn't a power of two; fold the rest
                while len(sums) > 1:
                    a, _ = sums.pop()
                    b, lv = sums.pop()
                    t2 = work.tile([128, SW], F32R, tag="pt2", bufs=6,
                                   name="pt2")
                    ew().tensor_add(t2, b, a)
                    sums.append([t2, lv + 1])

                # ---- finalize slice s: normalize + project + out ----
                def emit_otr(off, wdt, s=s, po=po):
                    otr = [work.tile([128, wdt], BF16, tag=f"otr{cc}",
                                     bufs=2, name=f"otr{cc}")
                           for cc in range(2)]
                    # po is PSUM: only DVE/ACT may read it
                    nc.vector.tensor_copy(otr[0], po[0][:, off:off + wdt])
                    nc.scalar.copy(otr[1], po[1][:, off:off + wdt])
                    return otr

                def make_finalize(off, wdt, otr, s=s, ptp=ptp, sums=sums,
                                  tail=tail, dma_engs=None):
                    state = {}

                    def fin_cc(cc):
                        sl2 = slice(s * SW + off, s * SW + off + wdt)
                        pf = pp.tile([128, wdt], F32, tag="sim", bufs=3,
                                     name="pf")
                        for ci in range(2):
                            mm(pf, w2t[ci][:, cc * 128:(cc + 1) * 128],
                               otr[ci], start=(ci == 0), stop=(ci == 1))
                        fo = work.tile([128, wdt], F32, tag="fo", bufs=2,
                                       name="fo")
                        nc.vector.tensor_mul(fo, pf, state["bc"])
                        fo2 = work.tile([128, wdt], F32, tag="fo2", bufs=2,
                                        name="fo2")
                        nc.vector.scalar_tensor_tensor(
                            fo2, xr[cc][:, sl2], bt[cc], fo,
                            op0=Add, op1=Add)
                        deng = dma_engs[cc] if dma_engs else nc.sync
                        deng.dma_start(out[cc * 128:(cc + 1) * 128, sl2], fo2)

                    def fin_a():
                        assert len(sums) == 1
                        pt16 = sums[0][0]
                        pb = pp.tile([128, wdt], F32, tag="aux", bufs=1,
                                     name="pb")
                        if not tail:
                            mm(pb, ones_sq, pt16[:, off:off + wdt],
                               start=True, stop=True)
                        else:
                            # tree covers pairs 0..13; last two pairs via
                            # DoubleRow all-ones colsum straight in PSUM
                            mm(pb, ones_sq, pt16[:, off:off + wdt],
                               start=True, stop=False)
                            mm(pb, ones_psq, ptp[-2][:, :, off:off + wdt],
                               start=False, stop=False, perf_mode=DR)
                            mm(pb, ones_psq, ptp[-1][:, :, off:off + wdt],
                               start=False, stop=True, perf_mode=DR)
                        bc = work.tile([128, wdt], F32, tag="bc", bufs=2,
                                       name="bc")
                        rsc = work.tile([128, wdt], F32, tag="rsc", bufs=2,
                                        name="rsc")
                        nc.vector.reciprocal_approx_accurate(bc, pb, rsc)
                        state["bc"] = bc
                        fin_cc(0)

                    def fin_b():
                        fin_cc(1)
                    return fin_a, fin_b

                if not tail:
                    otr = emit_otr(0, SW)
                    fa, fb = make_finalize(0, SW, otr)
                    deferred.append(fa)
                    deferred2.append(fb)
                else:
                    # kernel tail: two 256-col halves so the first half's
                    # projection/output overlaps the second's
                    hwd = SW // 2
                    otr0 = emit_otr(0, hwd)
                    otr1 = emit_otr(hwd, hwd)
                    a0, b0 = make_finalize(0, hwd, otr0,
                                           dma_engs=(nc.scalar, nc.sync))
                    a1, b1 = make_finalize(hwd, hwd, otr1,
                                           dma_engs=(nc.gpsimd, nc.scalar))
                    a0(); b0(); a1(); b1()

    nc.finalize()
    return nc


_NC_CACHE = None


def _get_nc():
    global _NC_CACHE
    if _NC_CACHE is None:
        _NC_CACHE = build_nc()
    return _NC_CACHE


def prepare_in_maps(x, w_qkv, w_out, b_out):
    x = np.asarray(x, dtype=np.float32)
    w_qkv = np.asarray(w_qkv, dtype=np.float32)
    w_out = np.asarray(w_out, dtype=np.float32)
    b_out = np.asarray(b_out, dtype=np.float32)

    import ml_dtypes
    bf16 = ml_dtypes.bfloat16
    f8e4 = ml_dtypes.float8_e4m3
    wq, wk, wv = w_qkv[:512], w_qkv[512:1024], w_qkv[1024:]
    M = (wq.T.astype(np.float64) @ wk.astype(np.float64)).astype(np.float32)
    W2 = (w_out.astype(np.float64) @ wv.astype(np.float64)).T.astype(np.float32)
    m_bf = np.ascontiguousarray(M).astype(bf16)
    w2_bf = np.ascontiguousarray(W2).astype(bf16)
    bout = np.ascontiguousarray(b_out.reshape(2, 128, 1))

    xr = x.reshape(B, C, N)
    in_maps = []
    for c in range(NCORES):
        b, h = divmod(c, 2)
        if h == 0:
            x_rot = xr[b]
        else:  # rotate so this core's query half sits in columns 0:HALF
            x_rot = np.concatenate([xr[b][:, HALF:], xr[b][:, :HALF]], axis=1)
        x_bf = x_rot.astype(bf16)
        # token-pair planes: xp[p, pair, plane, c] = xf[(pair*2+plane)*128+p, c]
        xf8 = np.ascontiguousarray(x_bf.T).astype(f8e4)          # [N, C]
        xp = np.ascontiguousarray(
            xf8.reshape(NPAIR, 2, 128, C).transpose(2, 0, 1, 3))  # [128,16,2,C]
        in_maps.append({
            "x_r": x_bf,
            "xp": xp,
            "m": m_bf,
            "w2": w2_bf,
            "bout": bout,
        })
    return in_maps


def postprocess(results):
    outs = [results[c]["out"] for c in range(NCORES)]
    full = np.stack([np.concatenate([outs[2 * b], outs[2 * b + 1]], axis=1)
                     for b in range(B)])               # [B, C, N]
    return full.reshape(B, C, 64, 64).astype(np.float32)


def kernel(x, w_qkv, w_out, b_out):
    in_maps = prepare_in_maps(x, w_qkv, w_out, b_out)
    res = run_bass_kernel_spmd(_get_nc(), in_maps, core_ids=list(range(NCORES)))
    return postprocess(res.results)
